# revision 16
# baseline (speedup 1.0000x reference)
"""Causal self-attention (B=4, N=2048, D=1024, H=16, hd=64) on 8 trn2 cores.

Sharding: core c -> (batch b = c//2, head-group hg = c%2 of 8 heads).
Each core computes, for its batch and its 8 heads (processed as 4 pairs):
  projT = (x[b] @ W_pair + bias)^T  via f32r matmuls (lhsT = W tiles, rhs = xT)
    laid out so rows are [K_h1|K_h2], [Q_h1|Q_h2], [V_h1|V_h2] (Q pre-scaled 1/8)
  S^T[k, q] = K^T.T @ Q^T  (row-packed pair of K=64 matmuls, causal blocks only)
  P^T = exp(S^T)  (ACT, psum->sbuf bf16), triangular mask on diagonal blocks
  outT[e, q] (+denominator row) = [V|1].T-stationary bf16 matmuls over k blocks
  out = outT[0:64] * (1/denom) broadcast; host transposes each head back.
"""

import sys

sys.path.insert(0, "/opt/trn_rl_repo")

import numpy as np
import ml_dtypes

BF16NP = ml_dtypes.bfloat16

B, N, D, H, HD = 4, 2048, 1024, 16, 64
NPAIR, DB, QC = 4, 8, 4  # head-pairs per core, 128-d-blocks, 512-q-chunks
SCALE = 1.0 / np.sqrt(HD)

_PROG_CACHE = {}


def build_program(rep=1, dbg=False, mmdt='f32r'):
    from concourse import bacc
    import concourse.bass as bass
    import concourse.mybir as mybir
    from concourse.tile import TileContext

    F32R, F32, BF = mybir.dt.float32r, mybir.dt.float32, mybir.dt.bfloat16
    MDT = {"f32r": F32R, "f32": F32, "bf16": BF}[mmdt]

    nc = bacc.Bacc("TRN2", target_bir_lowering=False)
    xt_d = nc.declare_dram_parameter("xt", [128, DB, N], MDT, isOutput=False)
    wt_d = nc.declare_dram_parameter("wt", [NPAIR, 128, DB, 384], MDT, isOutput=False)
    bias_d = nc.declare_dram_parameter("bias", [128, NPAIR, 3], F32, isOutput=False)
    mask_d = nc.declare_dram_parameter("mask", [128, 128], BF, isOutput=False)
    id_d = nc.declare_dram_parameter("ident", [128, 64], BF, isOutput=False)
    out_d = nc.declare_dram_parameter("outt", [2 * NPAIR, 64, N], F32, isOutput=True)
    if dbg:
        dkt_d = nc.declare_dram_parameter("dkt", [128, N], MDT, isOutput=True)
        dqt_d = nc.declare_dram_parameter("dqt", [128, N], MDT, isOutput=True)
        dvt_d = nc.declare_dram_parameter("dvt", [128, N], BF, isOutput=True)
        dva_d = nc.declare_dram_parameter("dva", [128, 16, 65], BF, isOutput=True)
        dpt_d = nc.declare_dram_parameter("dpt", [128, 1024], BF, isOutput=True)
        dpo_d = nc.declare_dram_parameter("dpo", [128, 512], F32, isOutput=True)
        drb_d = nc.declare_dram_parameter("drb", [64, 512], F32, isOutput=True)
        drc_d = nc.declare_dram_parameter("drc", [1, 512], F32, isOutput=True)

    with TileContext(nc) as tc:
        with (
            tc.tile_pool(name="big", bufs=1) as big,
            tc.tile_pool(name="wp", bufs=2) as wp,
            tc.tile_pool(name="projp", bufs=2) as projp,
            tc.tile_pool(name="attp", bufs=3) as attp,
            tc.tile_pool(name="psA", bufs=2, space="PSUM") as psA,
            tc.tile_pool(name="dr", bufs=4, space="DRAM") as dr,
        ):
            xt = big.tile([128, DB, N], MDT)
            for db in range(DB):
                nc.sync.dma_start(out=xt[:, db, :], in_=xt_d[:, db, :])
            mask = big.tile([128, 128], BF)
            nc.sync.dma_start(out=mask, in_=mask_d[:, :])
            ident = big.tile([128, 64], BF)
            nc.sync.dma_start(out=ident, in_=id_d[:, :])
            biasb = big.tile([128, NPAIR, 3], F32)
            nc.sync.dma_start(out=biasb, in_=bias_d[:, :, :])

            for _ in range(rep):
                for p in range(NPAIR):
                    w = wp.tile([128, DB, 384], MDT, tag="w")
                    nc.sync.dma_start(out=w, in_=wt_d[p])

                    # ---- projection: projT = (x @ Wp + b)^T, three 128-row m-blocks
                    kt = projp.tile([128, N], MDT, tag="kt")
                    qt = projp.tile([128, N], MDT, tag="qt")
                    vt = projp.tile([128, N], BF, tag="vt")
                    dests = [kt, qt, vt]
                    for m in range(3):
                        for n4 in range(QC):
                            pp = psA.tile([128, 512], F32, tag="proj")
                            for db in range(DB):
                                nc.tensor.matmul(
                                    pp,
                                    w[:, db, m * 128 : (m + 1) * 128],
                                    xt[:, db, n4 * 512 : (n4 + 1) * 512],
                                    start=(db == 0),
                                    stop=(db == DB - 1),
                                )
                            nc.vector.tensor_scalar_add(
                                dests[m][:, n4 * 512 : (n4 + 1) * 512],
                                pp,
                                biasb[:, p, m : m + 1],
                            )

                    # ---- V_aug: per head, [128(k), 16(kb), 64(e)+ones]
                    vaugs = []
                    for h2 in range(2):
                        va = attp.tile([128, 16, 65], BF, tag=f"va{h2}", bufs=2)
                        nc.vector.memset(va[:, :, 64:65], 1.0)
                        for kb in range(16):
                            pv = psA.tile([128, 64], BF, tag="proj")
                            nc.tensor.transpose(
                                pv,
                                vt[h2 * 64 : (h2 + 1) * 64, kb * 128 : (kb + 1) * 128],
                                ident[h2 * 64 : (h2 + 1) * 64, 0:64],
                                tile_position=(h2 * 64, 0),
                            )
                            nc.vector.tensor_copy(va[:, kb, 0:64], pv)
                        vaugs.append(va)

                    if dbg and p == 0:
                        nc.sync.dma_start(out=dkt_d[:, :], in_=kt)
                        nc.sync.dma_start(out=dqt_d[:, :], in_=qt)
                        nc.sync.dma_start(out=dvt_d[:, :], in_=vt)
                        nc.sync.dma_start(out=dva_d[:, :, :], in_=vaugs[0])

                    # ---- attention, per 512-wide q chunk
                    outts = [
                        attp.tile([64, N], F32, tag=f"outt{h2}", name=f"outt{h2}", bufs=2)
                        for h2 in range(2)
                    ]
                    for qc in range(QC):
                        po = [
                            psA.tile([128, 512], F32, tag="o", name=f"po{h2}")
                            for h2 in range(2)
                        ]
                        nkb = 4 * qc + 4
                        for kb in range(nkb):
                            q0 = 128 * max(0, kb - 4 * qc)
                            ps_s = psA.tile([128, 1024], F32, tag="s")
                            for h2 in range(2):
                                nc.tensor.matmul(
                                    ps_s[:, h2 * 512 + q0 : (h2 + 1) * 512],
                                    kt[h2 * 64 : (h2 + 1) * 64, kb * 128 : (kb + 1) * 128],
                                    qt[
                                        h2 * 64 : (h2 + 1) * 64,
                                        qc * 512 + q0 : (qc + 1) * 512,
                                    ],
                                    start=True,
                                    stop=True,
                                    tile_position=(h2 * 64, 0),
                                )
                            pt = attp.tile([128, 1024], BF, tag="pt")
                            sv = ps_s[:, :].rearrange("p (b w) -> p b w", b=2)
                            tv = pt[:, :].rearrange("p (b w) -> p b w", b=2)
                            nc.scalar.activation(
                                tv[:, :, q0:512],
                                sv[:, :, q0:512],
                                mybir.ActivationFunctionType.Exp,
                            )
                            if kb >= 4 * qc:  # diagonal block: triangular mask
                                for h2 in range(2):
                                    s = h2 * 512 + q0
                                    nc.vector.tensor_mul(
                                        pt[:, s : s + 128], pt[:, s : s + 128], mask
                                    )
                            if dbg and p == 0 and qc == 0 and kb == 0:
                                nc.sync.dma_start(out=dpt_d[:, :], in_=pt)
                            for h2 in range(2):
                                nc.tensor.matmul(
                                    po[h2][0:65, q0:512],
                                    vaugs[h2][:, kb, :],
                                    pt[:, h2 * 512 + q0 : (h2 + 1) * 512],
                                    start=(kb == 0),
                                    stop=(kb == nkb - 1),
                                )
                        if dbg and p == 0 and qc == 0:
                            dpo_sb = attp.tile([128, 512], F32, tag="dpo", name="dpo_sb", bufs=1)
                            nc.vector.tensor_copy(dpo_sb[0:65, :], po[0][0:65, :])
                            nc.sync.dma_start(out=dpo_d[:, :], in_=dpo_sb)
                        # normalize: out = po[0:64] / po[64]
                        for h2 in range(2):
                            den = attp.tile([65, 512], F32, tag="rec", bufs=2)
                            nc.vector.tensor_copy(den[64:65, :], po[h2][64:65, :])
                            recd = dr.tile([512], F32, tag="recd")
                            nc.sync.dma_start(
                                out=recd[:].rearrange("(a b) -> a b", a=1),
                                in_=den[64:65, :],
                            )
                            rb = attp.tile([64, 512], F32, tag="recb", bufs=2)
                            nc.sync.dma_start(
                                out=rb, in_=recd[:].partition_broadcast(64)
                            )
                            nc.vector.reciprocal_approx_fast(out=rb, in_=rb)
                            nc.vector.tensor_mul(
                                outts[h2][:, qc * 512 : (qc + 1) * 512],
                                po[h2][0:64, :],
                                rb,
                            )
                            if dbg and p == 0 and qc == 0 and h2 == 0:
                                nc.sync.dma_start(out=drb_d[:, :], in_=rb)
                                nc.sync.dma_start(out=drc_d[:, :], in_=rb[0:1, :])
                    for h2 in range(2):
                        nc.sync.dma_start(out=out_d[2 * p + h2], in_=outts[h2])

    nc.compile()
    return nc


def get_program(rep=1):
    if rep not in _PROG_CACHE:
        _PROG_CACHE[rep] = build_program(rep)
    return _PROG_CACHE[rep]


def prep_inputs(x, W, b, mmdt='f32r'):
    x = np.asarray(x, dtype=np.float32)
    W = np.asarray(W, dtype=np.float32)
    b = np.asarray(b, dtype=np.float32)
    mask = (np.arange(128)[:, None] <= np.arange(128)[None, :]).astype(BF16NP)
    ident = np.concatenate([np.eye(64), np.eye(64)], axis=0).astype(BF16NP)

    in_maps = []
    for c in range(8):
        bc, hg = divmod(c, 2)
        xt = np.ascontiguousarray(
            x[bc].T.reshape(DB, 128, N).transpose(1, 0, 2)
        )  # [128(dlow), DB, N]
        wt = np.empty((NPAIR, 128, DB, 384), np.float32)
        bias = np.empty((128, NPAIR, 3), np.float32)
        for p in range(NPAIR):
            g1, g2 = hg * 8 + 2 * p, hg * 8 + 2 * p + 1
            Wp = np.empty((D, 384), np.float32)
            Wp[:, 0:64] = W[g1, :, 0:64]
            Wp[:, 64:128] = W[g2, :, 0:64]
            Wp[:, 128:192] = W[g1, :, 64:128] * SCALE
            Wp[:, 192:256] = W[g2, :, 64:128] * SCALE
            Wp[:, 256:320] = W[g1, :, 128:192]
            Wp[:, 320:384] = W[g2, :, 128:192]
            wt[p] = Wp.reshape(DB, 128, 384).transpose(1, 0, 2)
            bias[0:64, p, 0] = b[g1, 0:64]
            bias[64:128, p, 0] = b[g2, 0:64]
            bias[0:64, p, 1] = b[g1, 64:128] * SCALE
            bias[64:128, p, 1] = b[g2, 64:128] * SCALE
            bias[0:64, p, 2] = b[g1, 128:192]
            bias[64:128, p, 2] = b[g2, 128:192]
        if mmdt == "bf16":
            xt, wt = xt.astype(BF16NP), wt.astype(BF16NP)
        in_maps.append(
            {"xt": xt, "wt": wt, "bias": bias, "mask": mask, "ident": ident}
        )
    return in_maps


def run(nc, in_maps):
    from concourse.bass_utils import run_bass_kernel_spmd

    return run_bass_kernel_spmd(nc, in_maps, list(range(8)))


def assemble(results):
    out = np.empty((B, N, D), np.float32)
    for c in range(8):
        bc, hg = divmod(c, 2)
        outt = results[c]["outt"]  # [8, 64, N]
        for hh in range(8):
            out[bc, :, hg * 512 + hh * 64 : hg * 512 + (hh + 1) * 64] = outt[hh].T
    return out


def kernel(x, W, b):
    nc = get_program(rep=1)
    res = run(nc, prep_inputs(x, W, b))
    return assemble(res.results)


# revision 18
# speedup vs baseline: 462.3818x; 462.3818x over previous
"""Causal self-attention (B=4, N=2048, D=1024, H=16, hd=64) on 8 trn2 cores.

Sharding: core c -> (batch b = c//2, head-group hg = c%2 of 8 heads).
Each core computes, for its batch and its 8 heads (processed as 4 pairs):
  projT = (x[b] @ W_pair + bias)^T  via f32r matmuls (lhsT = W tiles, rhs = xT)
    laid out so rows are [K_h1|K_h2], [Q_h1|Q_h2], [V_h1|V_h2] (Q pre-scaled 1/8)
  S^T[k, q] = K^T.T @ Q^T  (row-packed pair of K=64 matmuls, causal blocks only)
  P^T = exp(S^T)  (ACT, psum->sbuf bf16), triangular mask on diagonal blocks
  outT[e, q] (+denominator row) = [V|1].T-stationary bf16 matmuls over k blocks
  out = outT[0:64] * (1/denom) broadcast; host transposes each head back.
"""

import sys

sys.path.insert(0, "/opt/trn_rl_repo")

import numpy as np
import ml_dtypes

BF16NP = ml_dtypes.bfloat16

B, N, D, H, HD = 4, 2048, 1024, 16, 64
NPAIR, DB, QC = 4, 8, 4  # head-pairs per core, 128-d-blocks, 512-q-chunks
SCALE = 1.0 / np.sqrt(HD)

_PROG_CACHE = {}


def build_program(rep=1, dbg=False, mmdt='f32r', ablate='full'):
    from concourse import bacc
    import concourse.bass as bass
    import concourse.mybir as mybir
    from concourse.tile import TileContext

    F32R, F32, BF = mybir.dt.float32r, mybir.dt.float32, mybir.dt.bfloat16
    MDT = {"f32r": F32R, "f32": F32, "bf16": BF}[mmdt]

    nc = bacc.Bacc("TRN2", target_bir_lowering=False)
    xt_d = nc.declare_dram_parameter("xt", [128, DB, N], MDT, isOutput=False)
    wt_d = nc.declare_dram_parameter("wt", [NPAIR, 128, DB, 384], MDT, isOutput=False)
    bias_d = nc.declare_dram_parameter("bias", [128, NPAIR, 3], F32, isOutput=False)
    mask_d = nc.declare_dram_parameter("mask", [128, 128], BF, isOutput=False)
    id_d = nc.declare_dram_parameter("ident", [128, 64], BF, isOutput=False)
    out_d = nc.declare_dram_parameter("outt", [2 * NPAIR, 64, N], F32, isOutput=True)
    if dbg:
        dkt_d = nc.declare_dram_parameter("dkt", [128, N], MDT, isOutput=True)
        dqt_d = nc.declare_dram_parameter("dqt", [128, N], MDT, isOutput=True)
        dvt_d = nc.declare_dram_parameter("dvt", [128, N], BF, isOutput=True)
        dva_d = nc.declare_dram_parameter("dva", [128, 16, 65], BF, isOutput=True)
        dpt_d = nc.declare_dram_parameter("dpt", [128, 1024], BF, isOutput=True)
        dpo_d = nc.declare_dram_parameter("dpo", [128, 512], F32, isOutput=True)
        drb_d = nc.declare_dram_parameter("drb", [64, 512], F32, isOutput=True)
        drc_d = nc.declare_dram_parameter("drc", [1, 512], F32, isOutput=True)

    with TileContext(nc) as tc:
        with (
            tc.tile_pool(name="big", bufs=1) as big,
            tc.tile_pool(name="wp", bufs=2) as wp,
            tc.tile_pool(name="projp", bufs=2) as projp,
            tc.tile_pool(name="attp", bufs=3) as attp,
            tc.tile_pool(name="psA", bufs=2, space="PSUM") as psA,
            tc.tile_pool(name="dr", bufs=4, space="DRAM") as dr,
        ):
            xt = big.tile([128, DB, N], MDT)
            for db in range(DB):
                nc.sync.dma_start(out=xt[:, db, :], in_=xt_d[:, db, :])
            mask = big.tile([128, 128], BF)
            nc.sync.dma_start(out=mask, in_=mask_d[:, :])
            ident = big.tile([128, 64], BF)
            nc.sync.dma_start(out=ident, in_=id_d[:, :])
            biasb = big.tile([128, NPAIR, 3], F32)
            nc.sync.dma_start(out=biasb, in_=bias_d[:, :, :])

            for _ in range(rep):
                for p in range(NPAIR):
                    w = wp.tile([128, DB, 384], MDT, tag="w")
                    nc.sync.dma_start(out=w, in_=wt_d[p])

                    # ---- projection: projT = (x @ Wp + b)^T, three 128-row m-blocks
                    kt = projp.tile([128, N], MDT, tag="kt")
                    qt = projp.tile([128, N], MDT, tag="qt")
                    vt = projp.tile([128, N], BF, tag="vt")
                    dests = [kt, qt, vt]
                    for m in range(3):
                        for n4 in range(QC):
                            pp = psA.tile([128, 512], F32, tag="proj")
                            for db in range(DB):
                                nc.tensor.matmul(
                                    pp,
                                    w[:, db, m * 128 : (m + 1) * 128],
                                    xt[:, db, n4 * 512 : (n4 + 1) * 512],
                                    start=(db == 0),
                                    stop=(db == DB - 1),
                                )
                            nc.vector.tensor_scalar_add(
                                dests[m][:, n4 * 512 : (n4 + 1) * 512],
                                pp,
                                biasb[:, p, m : m + 1],
                            )

                    # ---- V_aug: per head, [128(k), 16(kb), 64(e)+ones]
                    vaugs = []
                    for h2 in range(2 if ablate not in ("proj",) else 0):
                        va = attp.tile([128, 16, 65], BF, tag=f"va{h2}", bufs=2)
                        nc.vector.memset(va[:, :, 64:65], 1.0)
                        for kb in range(16):
                            pv = psA.tile([128, 64], BF, tag="proj")
                            nc.tensor.transpose(
                                pv,
                                vt[h2 * 64 : (h2 + 1) * 64, kb * 128 : (kb + 1) * 128],
                                ident[h2 * 64 : (h2 + 1) * 64, 0:64],
                                tile_position=(h2 * 64, 0),
                            )
                            nc.vector.tensor_copy(va[:, kb, 0:64], pv)
                        vaugs.append(va)

                    if dbg and p == 0:
                        nc.sync.dma_start(out=dkt_d[:, :], in_=kt)
                        nc.sync.dma_start(out=dqt_d[:, :], in_=qt)
                        nc.sync.dma_start(out=dvt_d[:, :], in_=vt)
                        nc.sync.dma_start(out=dva_d[:, :, :], in_=vaugs[0])

                    # ---- attention, per 512-wide q chunk
                    outts = [
                        attp.tile([64, N], F32, tag=f"outt{h2}", name=f"outt{h2}", bufs=2)
                        for h2 in range(2)
                    ]
                    if ablate in ("proj", "noattn"):
                        for h2 in range(2):
                            nc.vector.tensor_copy(outts[h2], kt[h2 * 64 : (h2 + 1) * 64, :])
                        for h2 in range(2):
                            nc.sync.dma_start(out=out_d[2 * p + h2], in_=outts[h2])
                        continue
                    for qc in range(QC):
                        po = [
                            psA.tile([128, 512], F32, tag="o", name=f"po{h2}")
                            for h2 in range(2)
                        ]
                        nkb = 4 * qc + 4
                        for kb in range(nkb):
                            q0 = 128 * max(0, kb - 4 * qc)
                            ps_s = psA.tile([128, 1024], F32, tag="s")
                            for h2 in range(2):
                                nc.tensor.matmul(
                                    ps_s[:, h2 * 512 + q0 : (h2 + 1) * 512],
                                    kt[h2 * 64 : (h2 + 1) * 64, kb * 128 : (kb + 1) * 128],
                                    qt[
                                        h2 * 64 : (h2 + 1) * 64,
                                        qc * 512 + q0 : (qc + 1) * 512,
                                    ],
                                    start=True,
                                    stop=True,
                                    tile_position=(h2 * 64, 0),
                                )
                            pt = attp.tile([128, 1024], BF, tag="pt")
                            sv = ps_s[:, :].rearrange("p (b w) -> p b w", b=2)
                            tv = pt[:, :].rearrange("p (b w) -> p b w", b=2)
                            nc.scalar.activation(
                                tv[:, :, q0:512],
                                sv[:, :, q0:512],
                                mybir.ActivationFunctionType.Exp,
                            )
                            if kb >= 4 * qc:  # diagonal block: triangular mask
                                for h2 in range(2):
                                    s = h2 * 512 + q0
                                    nc.vector.tensor_mul(
                                        pt[:, s : s + 128], pt[:, s : s + 128], mask
                                    )
                            if dbg and p == 0 and qc == 0 and kb == 0:
                                nc.sync.dma_start(out=dpt_d[:, :], in_=pt)
                            for h2 in range(2):
                                nc.tensor.matmul(
                                    po[h2][0:65, q0:512],
                                    vaugs[h2][:, kb, :],
                                    pt[:, h2 * 512 + q0 : (h2 + 1) * 512],
                                    start=(kb == 0),
                                    stop=(kb == nkb - 1),
                                )
                        if dbg and p == 0 and qc == 0:
                            dpo_sb = attp.tile([128, 512], F32, tag="dpo", name="dpo_sb", bufs=1)
                            nc.vector.tensor_copy(dpo_sb[0:65, :], po[0][0:65, :])
                            nc.sync.dma_start(out=dpo_d[:, :], in_=dpo_sb)
                        # normalize: out = po[0:64] / po[64]
                        if ablate == "noepi":
                            for h2 in range(2):
                                nc.vector.tensor_copy(
                                    outts[h2][:, qc * 512 : (qc + 1) * 512],
                                    po[h2][0:64, :],
                                )
                            continue
                        for h2 in range(2):
                            den = attp.tile([65, 512], F32, tag="rec", bufs=2)
                            nc.vector.tensor_copy(den[64:65, :], po[h2][64:65, :])
                            recd = dr.tile([512], F32, tag="recd")
                            nc.sync.dma_start(
                                out=recd[:].rearrange("(a b) -> a b", a=1),
                                in_=den[64:65, :],
                            )
                            rb = attp.tile([64, 512], F32, tag="recb", bufs=2)
                            nc.sync.dma_start(
                                out=rb, in_=recd[:].partition_broadcast(64)
                            )
                            nc.vector.reciprocal_approx_fast(out=rb, in_=rb)
                            nc.vector.tensor_mul(
                                outts[h2][:, qc * 512 : (qc + 1) * 512],
                                po[h2][0:64, :],
                                rb,
                            )
                            if dbg and p == 0 and qc == 0 and h2 == 0:
                                nc.sync.dma_start(out=drb_d[:, :], in_=rb)
                                nc.sync.dma_start(out=drc_d[:, :], in_=rb[0:1, :])
                    for h2 in range(2):
                        nc.sync.dma_start(out=out_d[2 * p + h2], in_=outts[h2])

    nc.compile()
    return nc


def get_program(rep=1):
    if rep not in _PROG_CACHE:
        _PROG_CACHE[rep] = build_program(rep)
    return _PROG_CACHE[rep]


def prep_inputs(x, W, b, mmdt='f32r'):
    x = np.asarray(x, dtype=np.float32)
    W = np.asarray(W, dtype=np.float32)
    b = np.asarray(b, dtype=np.float32)
    mask = (np.arange(128)[:, None] <= np.arange(128)[None, :]).astype(BF16NP)
    ident = np.concatenate([np.eye(64), np.eye(64)], axis=0).astype(BF16NP)

    in_maps = []
    for c in range(8):
        bc, hg = divmod(c, 2)
        xt = np.ascontiguousarray(
            x[bc].T.reshape(DB, 128, N).transpose(1, 0, 2)
        )  # [128(dlow), DB, N]
        wt = np.empty((NPAIR, 128, DB, 384), np.float32)
        bias = np.empty((128, NPAIR, 3), np.float32)
        for p in range(NPAIR):
            g1, g2 = hg * 8 + 2 * p, hg * 8 + 2 * p + 1
            Wp = np.empty((D, 384), np.float32)
            Wp[:, 0:64] = W[g1, :, 0:64]
            Wp[:, 64:128] = W[g2, :, 0:64]
            Wp[:, 128:192] = W[g1, :, 64:128] * SCALE
            Wp[:, 192:256] = W[g2, :, 64:128] * SCALE
            Wp[:, 256:320] = W[g1, :, 128:192]
            Wp[:, 320:384] = W[g2, :, 128:192]
            wt[p] = Wp.reshape(DB, 128, 384).transpose(1, 0, 2)
            bias[0:64, p, 0] = b[g1, 0:64]
            bias[64:128, p, 0] = b[g2, 0:64]
            bias[0:64, p, 1] = b[g1, 64:128] * SCALE
            bias[64:128, p, 1] = b[g2, 64:128] * SCALE
            bias[0:64, p, 2] = b[g1, 128:192]
            bias[64:128, p, 2] = b[g2, 128:192]
        if mmdt == "bf16":
            xt, wt = xt.astype(BF16NP), wt.astype(BF16NP)
        in_maps.append(
            {"xt": xt, "wt": wt, "bias": bias, "mask": mask, "ident": ident}
        )
    return in_maps


def run(nc, in_maps):
    from concourse.bass_utils import run_bass_kernel_spmd

    return run_bass_kernel_spmd(nc, in_maps, list(range(8)))


class Runner:
    """Persistent PJRT executable for an nc program: loads the NEFF once and
    reuses it across calls (run_bass_via_pjrt reloads per call)."""

    def __init__(self, nc, n_cores=8):
        import jax
        import numpy as np
        from jax.sharding import Mesh, PartitionSpec
        from jax.experimental.shard_map import shard_map
        import concourse.mybir as mybir
        from concourse import bass2jax

        bass2jax.install_neuronx_cc_hook()
        self.n_cores = n_cores
        partition_name = (
            nc.partition_id_tensor.name if nc.partition_id_tensor else None
        )
        in_names, out_names, out_avals, zero_outs = [], [], [], []
        for alloc in nc.m.functions[0].allocations:
            if not isinstance(alloc, mybir.MemoryLocationSet):
                continue
            name = alloc.memorylocations[0].name
            if alloc.kind == "ExternalInput":
                if name != partition_name:
                    in_names.append(name)
            elif alloc.kind == "ExternalOutput":
                shape = tuple(alloc.tensor_shape)
                dtype = mybir.dt.np(alloc.dtype)
                out_names.append(name)
                out_avals.append(jax.core.ShapedArray(shape, dtype))
                zero_outs.append(np.zeros(shape, dtype))
        n_params = len(in_names)
        all_in_names = list(in_names) + list(out_names)
        if partition_name is not None:
            all_in_names.append(partition_name)

        def _body(*args):
            operands = list(args)
            if partition_name is not None:
                operands.append(bass2jax.partition_id_tensor())
            outs = bass2jax._bass_exec_p.bind(
                *operands,
                out_avals=tuple(out_avals),
                in_names=tuple(all_in_names),
                out_names=tuple(out_names),
                lowering_input_output_aliases=(),
                sim_require_finite=True,
                sim_require_nnan=True,
                nc=nc,
            )
            return tuple(outs)

        devices = jax.devices()[:n_cores]
        mesh = Mesh(np.asarray(devices), ("core",))
        in_specs = (PartitionSpec("core"),) * (n_params + len(out_names))
        out_specs = (PartitionSpec("core"),) * len(out_names)
        self._fn = jax.jit(
            shard_map(
                _body,
                mesh=mesh,
                in_specs=in_specs,
                out_specs=out_specs,
                check_rep=False,
            ),
            keep_unused=True,
        )
        self.in_names, self.out_names = in_names, out_names
        self.out_avals, self.zero_outs = out_avals, zero_outs
        self.n_params = n_params
        self._jax = jax

    def put_inputs(self, in_maps):
        import numpy as np

        concat_in = [
            np.concatenate([np.asarray(m[n]) for m in in_maps], axis=0)
            for n in self.in_names
        ]
        concat_zeros = [
            np.zeros((self.n_cores * z.shape[0], *z.shape[1:]), z.dtype)
            for z in self.zero_outs
        ]
        return [self._jax.device_put(a) for a in concat_in + concat_zeros]

    def execute(self, dev_args):
        outs = self._fn(*dev_args)
        self._jax.block_until_ready(outs)
        return outs

    def run(self, in_maps):
        import numpy as np

        outs = self.execute(self.put_inputs(in_maps))
        return [
            {
                n: np.asarray(outs[i]).reshape(
                    self.n_cores, *self.out_avals[i].shape
                )[c]
                for i, n in enumerate(self.out_names)
            }
            for c in range(self.n_cores)
        ]


def assemble(results):
    out = np.empty((B, N, D), np.float32)
    for c in range(8):
        bc, hg = divmod(c, 2)
        outt = results[c]["outt"]  # [8, 64, N]
        for hh in range(8):
            out[bc, :, hg * 512 + hh * 64 : hg * 512 + (hh + 1) * 64] = outt[hh].T
    return out


def kernel(x, W, b):
    nc = get_program(rep=1)
    res = run(nc, prep_inputs(x, W, b))
    return assemble(res.results)


# revision 19
# speedup vs baseline: 540.2922x; 1.1685x over previous
"""Causal self-attention (B=4, N=2048, D=1024, H=16, hd=64) on 8 trn2 cores.

Sharding: core c -> (batch b = c//2, head-group hg = c%2 of 8 heads).
Each core computes, for its batch and its 8 heads (processed as 4 pairs):
  projT = (x[b] @ W_pair + bias)^T  via f32r matmuls (lhsT = W tiles, rhs = xT)
    laid out so rows are [K_h1|K_h2], [Q_h1|Q_h2], [V_h1|V_h2] (Q pre-scaled 1/8)
  S^T[k, q] = K^T.T @ Q^T  (row-packed pair of K=64 matmuls, causal blocks only)
  P^T = exp(S^T)  (ACT, psum->sbuf bf16), triangular mask on diagonal blocks
  outT[e, q] (+denominator row) = [V|1].T-stationary bf16 matmuls over k blocks
  out = outT[0:64] * (1/denom) broadcast; host transposes each head back.
"""

import sys

sys.path.insert(0, "/opt/trn_rl_repo")

import numpy as np
import ml_dtypes

BF16NP = ml_dtypes.bfloat16

B, N, D, H, HD = 4, 2048, 1024, 16, 64
NPAIR, DB, QC = 4, 8, 4  # head-pairs per core, 128-d-blocks, 512-q-chunks
SCALE = 1.0 / np.sqrt(HD)

_PROG_CACHE = {}


def build_program(rep=1, dbg=False, mmdt='f32r', ablate='full'):
    from concourse import bacc
    import concourse.bass as bass
    import concourse.mybir as mybir
    from concourse.tile import TileContext

    F32R, F32, BF = mybir.dt.float32r, mybir.dt.float32, mybir.dt.bfloat16
    MDT = {"f32r": F32R, "f32": F32, "bf16": BF}[mmdt]

    nc = bacc.Bacc("TRN2", target_bir_lowering=False)
    xt_d = nc.declare_dram_parameter("xt", [128, DB, N], MDT, isOutput=False)
    wt_d = nc.declare_dram_parameter("wt", [NPAIR, 128, DB, 384], MDT, isOutput=False)
    bias_d = nc.declare_dram_parameter("bias", [128, NPAIR, 3], F32, isOutput=False)
    mask_d = nc.declare_dram_parameter("mask", [128, 128], BF, isOutput=False)
    id_d = nc.declare_dram_parameter("ident", [128, 64], BF, isOutput=False)
    out_d = nc.declare_dram_parameter("outt", [2 * NPAIR, 64, N], F32, isOutput=True)
    if dbg:
        dkt_d = nc.declare_dram_parameter("dkt", [128, N], MDT, isOutput=True)
        dqt_d = nc.declare_dram_parameter("dqt", [128, N], MDT, isOutput=True)
        dvt_d = nc.declare_dram_parameter("dvt", [128, N], BF, isOutput=True)
        dva_d = nc.declare_dram_parameter("dva", [128, 16, 65], BF, isOutput=True)
        dpt_d = nc.declare_dram_parameter("dpt", [128, 1024], BF, isOutput=True)
        dpo_d = nc.declare_dram_parameter("dpo", [128, 512], F32, isOutput=True)
        drb_d = nc.declare_dram_parameter("drb", [64, 512], F32, isOutput=True)
        drc_d = nc.declare_dram_parameter("drc", [1, 512], F32, isOutput=True)

    with TileContext(nc) as tc:
        with (
            tc.tile_pool(name="big", bufs=1) as big,
            tc.tile_pool(name="wp", bufs=2) as wp,
            tc.tile_pool(name="projp", bufs=2) as projp,
            tc.tile_pool(name="attp", bufs=3) as attp,
            tc.tile_pool(name="psA", bufs=2, space="PSUM") as psA,
            tc.tile_pool(name="dr", bufs=4, space="DRAM") as dr,
        ):
            xt = big.tile([128, DB, N], MDT)
            for db in range(DB):
                nc.sync.dma_start(out=xt[:, db, :], in_=xt_d[:, db, :])
            mask = big.tile([128, 128], BF)
            nc.sync.dma_start(out=mask, in_=mask_d[:, :])
            ident = big.tile([128, 64], BF)
            nc.sync.dma_start(out=ident, in_=id_d[:, :])
            biasb = big.tile([128, NPAIR, 3], F32)
            nc.sync.dma_start(out=biasb, in_=bias_d[:, :, :])

            for _ in range(rep):
                for p in range(NPAIR):
                    w = wp.tile([128, DB, 384], MDT, tag="w")
                    nc.sync.dma_start(out=w, in_=wt_d[p])

                    # ---- projection: projT = (x @ Wp + b)^T, three 128-row m-blocks
                    kt = projp.tile([128, N], MDT, tag="kt")
                    qt = projp.tile([128, N], MDT, tag="qt")
                    vt = projp.tile([128, N], BF, tag="vt")
                    dests = [kt, qt, vt]
                    for m in range(3):
                        for n4 in range(QC):
                            pp = psA.tile([128, 512], F32, tag="proj")
                            for db in range(DB):
                                nc.tensor.matmul(
                                    pp,
                                    w[:, db, m * 128 : (m + 1) * 128],
                                    xt[:, db, n4 * 512 : (n4 + 1) * 512],
                                    start=(db == 0),
                                    stop=(db == DB - 1),
                                )
                            nc.vector.tensor_scalar_add(
                                dests[m][:, n4 * 512 : (n4 + 1) * 512],
                                pp,
                                biasb[:, p, m : m + 1],
                            )

                    # ---- V_aug: per head, [128(k), 16(kb), 64(e)+ones]
                    vaugs = []
                    for h2 in range(2 if ablate not in ("proj",) else 0):
                        va = attp.tile([128, 16, 65], BF, tag=f"va{h2}", bufs=2)
                        nc.vector.memset(va[:, :, 64:65], 1.0)
                        for kb in range(16):
                            pv = psA.tile([128, 64], BF, tag="proj")
                            nc.tensor.transpose(
                                pv,
                                vt[h2 * 64 : (h2 + 1) * 64, kb * 128 : (kb + 1) * 128],
                                ident[h2 * 64 : (h2 + 1) * 64, 0:64],
                                tile_position=(h2 * 64, 0),
                            )
                            nc.vector.tensor_copy(va[:, kb, 0:64], pv)
                        vaugs.append(va)

                    if dbg and p == 0:
                        nc.sync.dma_start(out=dkt_d[:, :], in_=kt)
                        nc.sync.dma_start(out=dqt_d[:, :], in_=qt)
                        nc.sync.dma_start(out=dvt_d[:, :], in_=vt)
                        nc.sync.dma_start(out=dva_d[:, :, :], in_=vaugs[0])

                    # ---- attention, per 512-wide q chunk
                    outts = [
                        attp.tile([64, N], F32, tag=f"outt{h2}", name=f"outt{h2}", bufs=2)
                        for h2 in range(2)
                    ]
                    if ablate in ("proj", "noattn"):
                        for h2 in range(2):
                            nc.vector.tensor_copy(outts[h2], kt[h2 * 64 : (h2 + 1) * 64, :])
                        for h2 in range(2):
                            nc.sync.dma_start(out=out_d[2 * p + h2], in_=outts[h2])
                        continue
                    for qc in range(QC):
                        po = [
                            psA.tile([128, 512], F32, tag="o", name=f"po{h2}")
                            for h2 in range(2)
                        ]
                        nkb = 4 * qc + 4
                        for kb in range(nkb):
                            q0 = 128 * max(0, kb - 4 * qc)
                            ps_s = psA.tile([128, 1024], F32, tag="s")
                            for h2 in range(2):
                                nc.tensor.matmul(
                                    ps_s[:, h2 * 512 + q0 : (h2 + 1) * 512],
                                    kt[h2 * 64 : (h2 + 1) * 64, kb * 128 : (kb + 1) * 128],
                                    qt[
                                        h2 * 64 : (h2 + 1) * 64,
                                        qc * 512 + q0 : (qc + 1) * 512,
                                    ],
                                    start=True,
                                    stop=True,
                                    tile_position=(h2 * 64, 0),
                                )
                            pt = attp.tile([128, 1024], BF, tag="pt")
                            sv = ps_s[:, :].rearrange("p (b w) -> p b w", b=2)
                            tv = pt[:, :].rearrange("p (b w) -> p b w", b=2)
                            nc.scalar.activation(
                                tv[:, :, q0:512],
                                sv[:, :, q0:512],
                                mybir.ActivationFunctionType.Exp,
                            )
                            if kb >= 4 * qc:  # diagonal block: triangular mask
                                for h2 in range(2):
                                    s = h2 * 512 + q0
                                    nc.vector.tensor_mul(
                                        pt[:, s : s + 128], pt[:, s : s + 128], mask
                                    )
                            if dbg and p == 0 and qc == 0 and kb == 0:
                                nc.sync.dma_start(out=dpt_d[:, :], in_=pt)
                            for h2 in range(2):
                                nc.tensor.matmul(
                                    po[h2][0:65, q0:512],
                                    vaugs[h2][:, kb, :],
                                    pt[:, h2 * 512 + q0 : (h2 + 1) * 512],
                                    start=(kb == 0),
                                    stop=(kb == nkb - 1),
                                )
                        if dbg and p == 0 and qc == 0:
                            dpo_sb = attp.tile([128, 512], F32, tag="dpo", name="dpo_sb", bufs=1)
                            nc.vector.tensor_copy(dpo_sb[0:65, :], po[0][0:65, :])
                            nc.sync.dma_start(out=dpo_d[:, :], in_=dpo_sb)
                        # normalize: out = po[0:64] / po[64]
                        if ablate == "noepi":
                            for h2 in range(2):
                                nc.vector.tensor_copy(
                                    outts[h2][:, qc * 512 : (qc + 1) * 512],
                                    po[h2][0:64, :],
                                )
                            continue
                        for h2 in range(2):
                            osb = attp.tile([65, 512], F32, tag="osb", bufs=3)
                            nc.vector.tensor_copy(osb, po[h2][0:65, :])
                            recd = dr.tile([512], F32, tag="recd")
                            nc.sync.dma_start(
                                out=recd[:].rearrange("(a b) -> a b", a=1),
                                in_=osb[64:65, :],
                            )
                            rb = attp.tile([64, 512], F32, tag="recb", bufs=2)
                            nc.sync.dma_start(
                                out=rb, in_=recd[:].partition_broadcast(64)
                            )
                            nc.vector.reciprocal_approx_fast(out=rb, in_=rb)
                            nc.vector.tensor_mul(
                                outts[h2][:, qc * 512 : (qc + 1) * 512],
                                osb[0:64, :],
                                rb,
                            )
                            if dbg and p == 0 and qc == 0 and h2 == 0:
                                nc.sync.dma_start(out=drb_d[:, :], in_=rb)
                                nc.sync.dma_start(out=drc_d[:, :], in_=rb[0:1, :])
                    for h2 in range(2):
                        nc.sync.dma_start(out=out_d[2 * p + h2], in_=outts[h2])

    nc.compile()
    return nc


def get_program(rep=1):
    if rep not in _PROG_CACHE:
        _PROG_CACHE[rep] = build_program(rep)
    return _PROG_CACHE[rep]


def prep_inputs(x, W, b, mmdt='f32r'):
    x = np.asarray(x, dtype=np.float32)
    W = np.asarray(W, dtype=np.float32)
    b = np.asarray(b, dtype=np.float32)
    mask = (np.arange(128)[:, None] <= np.arange(128)[None, :]).astype(BF16NP)
    ident = np.concatenate([np.eye(64), np.eye(64)], axis=0).astype(BF16NP)

    in_maps = []
    for c in range(8):
        bc, hg = divmod(c, 2)
        xt = np.ascontiguousarray(
            x[bc].T.reshape(DB, 128, N).transpose(1, 0, 2)
        )  # [128(dlow), DB, N]
        wt = np.empty((NPAIR, 128, DB, 384), np.float32)
        bias = np.empty((128, NPAIR, 3), np.float32)
        for p in range(NPAIR):
            g1, g2 = hg * 8 + 2 * p, hg * 8 + 2 * p + 1
            Wp = np.empty((D, 384), np.float32)
            Wp[:, 0:64] = W[g1, :, 0:64]
            Wp[:, 64:128] = W[g2, :, 0:64]
            Wp[:, 128:192] = W[g1, :, 64:128] * SCALE
            Wp[:, 192:256] = W[g2, :, 64:128] * SCALE
            Wp[:, 256:320] = W[g1, :, 128:192]
            Wp[:, 320:384] = W[g2, :, 128:192]
            wt[p] = Wp.reshape(DB, 128, 384).transpose(1, 0, 2)
            bias[0:64, p, 0] = b[g1, 0:64]
            bias[64:128, p, 0] = b[g2, 0:64]
            bias[0:64, p, 1] = b[g1, 64:128] * SCALE
            bias[64:128, p, 1] = b[g2, 64:128] * SCALE
            bias[0:64, p, 2] = b[g1, 128:192]
            bias[64:128, p, 2] = b[g2, 128:192]
        if mmdt == "bf16":
            xt, wt = xt.astype(BF16NP), wt.astype(BF16NP)
        in_maps.append(
            {"xt": xt, "wt": wt, "bias": bias, "mask": mask, "ident": ident}
        )
    return in_maps


def run(nc, in_maps):
    from concourse.bass_utils import run_bass_kernel_spmd

    return run_bass_kernel_spmd(nc, in_maps, list(range(8)))


class Runner:
    """Persistent PJRT executable for an nc program: loads the NEFF once and
    reuses it across calls (run_bass_via_pjrt reloads per call)."""

    def __init__(self, nc, n_cores=8):
        import jax
        import numpy as np
        from jax.sharding import Mesh, PartitionSpec
        from jax.experimental.shard_map import shard_map
        import concourse.mybir as mybir
        from concourse import bass2jax

        bass2jax.install_neuronx_cc_hook()
        self.n_cores = n_cores
        partition_name = (
            nc.partition_id_tensor.name if nc.partition_id_tensor else None
        )
        in_names, out_names, out_avals, zero_outs = [], [], [], []
        for alloc in nc.m.functions[0].allocations:
            if not isinstance(alloc, mybir.MemoryLocationSet):
                continue
            name = alloc.memorylocations[0].name
            if alloc.kind == "ExternalInput":
                if name != partition_name:
                    in_names.append(name)
            elif alloc.kind == "ExternalOutput":
                shape = tuple(alloc.tensor_shape)
                dtype = mybir.dt.np(alloc.dtype)
                out_names.append(name)
                out_avals.append(jax.core.ShapedArray(shape, dtype))
                zero_outs.append(np.zeros(shape, dtype))
        n_params = len(in_names)
        all_in_names = list(in_names) + list(out_names)
        if partition_name is not None:
            all_in_names.append(partition_name)

        def _body(*args):
            operands = list(args)
            if partition_name is not None:
                operands.append(bass2jax.partition_id_tensor())
            outs = bass2jax._bass_exec_p.bind(
                *operands,
                out_avals=tuple(out_avals),
                in_names=tuple(all_in_names),
                out_names=tuple(out_names),
                lowering_input_output_aliases=(),
                sim_require_finite=True,
                sim_require_nnan=True,
                nc=nc,
            )
            return tuple(outs)

        devices = jax.devices()[:n_cores]
        mesh = Mesh(np.asarray(devices), ("core",))
        in_specs = (PartitionSpec("core"),) * (n_params + len(out_names))
        out_specs = (PartitionSpec("core"),) * len(out_names)
        self._fn = jax.jit(
            shard_map(
                _body,
                mesh=mesh,
                in_specs=in_specs,
                out_specs=out_specs,
                check_rep=False,
            ),
            keep_unused=True,
        )
        self.in_names, self.out_names = in_names, out_names
        self.out_avals, self.zero_outs = out_avals, zero_outs
        self.n_params = n_params
        self._jax = jax

    def put_inputs(self, in_maps):
        import numpy as np

        concat_in = [
            np.concatenate([np.asarray(m[n]) for m in in_maps], axis=0)
            for n in self.in_names
        ]
        concat_zeros = [
            np.zeros((self.n_cores * z.shape[0], *z.shape[1:]), z.dtype)
            for z in self.zero_outs
        ]
        return [self._jax.device_put(a) for a in concat_in + concat_zeros]

    def execute(self, dev_args):
        outs = self._fn(*dev_args)
        self._jax.block_until_ready(outs)
        return outs

    def run(self, in_maps):
        import numpy as np

        outs = self.execute(self.put_inputs(in_maps))
        return [
            {
                n: np.asarray(outs[i]).reshape(
                    self.n_cores, *self.out_avals[i].shape
                )[c]
                for i, n in enumerate(self.out_names)
            }
            for c in range(self.n_cores)
        ]


def assemble(results):
    out = np.empty((B, N, D), np.float32)
    for c in range(8):
        bc, hg = divmod(c, 2)
        outt = results[c]["outt"]  # [8, 64, N]
        for hh in range(8):
            out[bc, :, hg * 512 + hh * 64 : hg * 512 + (hh + 1) * 64] = outt[hh].T
    return out


def kernel(x, W, b):
    nc = get_program(rep=1)
    res = run(nc, prep_inputs(x, W, b))
    return assemble(res.results)
